# revision 1
# baseline (speedup 1.0000x reference)
"""Trainium2 Bass kernel for nn_ContinuousDiffusion (GNN message passing).

Algorithm (reference):
    h = tanh(gat_out @ W_in + b_in)
    deg = clip(segment_sum(ones, dst), 1)
    for 12 steps:
        agg = segment_sum(h[src], dst) / deg
        diff = tanh(gelu(agg @ W1 + b1) @ W2 + b2)
        h = h + (diff - relu(clearance)*h) * dt

Distribution: nodes sharded by dst across 8 cores (6250 + 22 pad rows each).
Each Euler step does an 8-core AllGather of h into a per-core full table,
then a per-core dma_gather of h[src] for its (dst-sorted, slot-padded)
edges, a DVE slot-sum per 128-node block, and a dense MLP on PE/ACT.

Because dma_gather indices are int16, the 50176-row table is addressed via
two gather calls: "lo" edges (src on cores 0-4, rows 0..31359) and "hi"
edges (src on cores 5-7, rows relative to base 31360).
"""

import sys
import numpy as np

sys.path.insert(0, "/opt/trn_rl_repo")

N_NODES = 50000
N_EDGES = 800000
IN_DIM = 128
HID = 128
TIME = 6.0
STEPS = 12
N_CORES = 8
CH = N_NODES // N_CORES          # 6250 real nodes per core
NB = (CH + 127) // 128           # 49 blocks
CHP = NB * 128                   # 6272 padded rows per core
NPAD = CHP - CH                  # 22 pad rows (placed first in each chunk)
TAB = N_CORES * CHP              # 50176 table rows
LO_CORES = 5                     # cores 0..4 are the "lo" half of the table
LO_BASE = 0
HI_BASE = LO_CORES * CHP         # 31360; rows >= HI_BASE use the hi gather
DT = TIME / STEPS

# zero rows (h of pad rows is kept at 0): core0 pad row 0 for lo,
# core7 pad row for hi.
Z_LO = 0
Z_HI_ABS = 7 * CHP               # 43904
assert HI_BASE < 2**15 and (TAB - HI_BASE) < 2**15 and Z_HI_ABS - HI_BASE < 2**15


def _wrap_idx(idx_linear):
    """Linear int16 index list -> [128, n/16] wrapped+replicated layout."""
    n = idx_linear.shape[0]
    assert n % 16 == 0
    w = idx_linear.reshape(n // 16, 16).T.astype(np.int16)  # [16, n/16]
    return np.tile(w, (8, 1))  # replicate across the 8 groups of 16 partitions


def build_layout(edge_index):
    """Host-side graph layout. Returns a dict with everything the device
    program and the output reassembly need. Pure numpy, O(E log E)."""
    src = np.asarray(edge_index[0], dtype=np.int64)
    dst = np.asarray(edge_index[1], dtype=np.int64)

    core_of = dst // CH                     # owner core of each edge (by dst)
    src_core = src // CH

    deg = np.bincount(dst, minlength=N_NODES).astype(np.int64)

    # ---- final within-core node order: pads first, then real nodes sorted
    # by total degree (helps keep per-block slot maxima tight).
    # final_pos[n] in [0, CHP) within its core.
    final_pos = np.empty(N_NODES, dtype=np.int64)
    for c in range(N_CORES):
        lo, hi = c * CH, (c + 1) * CH
        order = np.argsort(deg[lo:hi], kind="stable")  # local ids sorted asc
        final_pos[lo + order] = NPAD + np.arange(CH)
    abs_row = (np.arange(N_NODES) // CH) * CHP + final_pos  # table row of node

    is_lo_edge = src_core < LO_CORES
    src_row = abs_row[src]                  # table row of each edge's source

    # ---- per-core slot layout ------------------------------------------
    # For core c, node at final position q (in [NPAD, CHP)), its edges are
    # split lo/hi.  Block b holds q in [128b, 128(b+1)).  T_LO[b]/T_HI[b]
    # are common across cores (max) so a single SPMD program works.
    d_lo = np.bincount(dst[is_lo_edge], minlength=N_NODES)
    d_hi = np.bincount(dst[~is_lo_edge], minlength=N_NODES)

    # per (core, block) maxima
    q_of = final_pos  # within-core position of each node
    blk_of = q_of // 128
    T_LO = np.zeros(NB, dtype=np.int64)
    T_HI = np.zeros(NB, dtype=np.int64)
    for arr, T in ((d_lo, T_LO), (d_hi, T_HI)):
        # max over all nodes mapped to block b (cores collapsed)
        np.maximum.at(T, blk_of, arr)

    lo_cols = int(T_LO.sum())
    hi_cols = int(T_HI.sum())

    # slot bases per block (column offset within the phase's slot space)
    lo_base = np.concatenate([[0], np.cumsum(T_LO)[:-1]])
    hi_base = np.concatenate([[0], np.cumsum(T_HI)[:-1]])

    NLO = 128 * lo_cols
    NHI = 128 * hi_cols

    # ---- per-core index arrays -----------------------------------------
    # Sort edges by (core, block(q), q) then assign slot t by running count.
    idx_lo_all = np.full((N_CORES, NLO), Z_LO, dtype=np.int64)
    idx_hi_all = np.full((N_CORES, NHI), Z_HI_ABS - HI_BASE, dtype=np.int64)

    q_edge = q_of[dst]                      # within-core position of dst
    for half, idx_all, base_tab, T_arr, base_arr in (
        (0, idx_lo_all, LO_BASE, T_LO, lo_base),
        (1, idx_hi_all, HI_BASE, T_HI, hi_base),
    ):
        m = is_lo_edge if half == 0 else ~is_lo_edge
        ec, eq, esrow = core_of[m], q_edge[m], src_row[m]
        order = np.lexsort((eq, ec))
        ec, eq, esrow = ec[order], eq[order], esrow[order]
        # slot t = occurrence counter of (core, q) in sorted order
        key = ec * CHP + eq
        t = np.arange(key.size) - np.concatenate(
            [[0], np.cumsum(np.bincount(key, minlength=N_CORES * CHP))]
        )[key]
        b = eq // 128
        p = eq % 128
        # linear slot position within the phase: (base[b] + t)*128 + p
        pos = (base_arr[b] + t) * 128 + p
        idx_all[ec, pos] = esrow - base_tab
        assert (t < T_arr[b]).all()

    inv_deg = np.zeros((N_CORES, CHP), dtype=np.float32)
    for c in range(N_CORES):
        lo, hi = c * CH, (c + 1) * CH
        ideg = 1.0 / np.maximum(deg[lo:hi], 1).astype(np.float32)
        inv_deg[c, final_pos[lo:hi]] = ideg

    return dict(
        T_LO=T_LO, T_HI=T_HI, lo_base=lo_base, hi_base=hi_base,
        NLO=NLO, NHI=NHI,
        idx_lo=idx_lo_all.astype(np.int16),
        idx_hi=idx_hi_all.astype(np.int16),
        inv_deg=inv_deg, abs_row=abs_row, final_pos=final_pos, deg=deg,
    )


# ---------------------------------------------------------------------------
# device program
# ---------------------------------------------------------------------------

GATHER_BUDGET = 64   # max slot-columns (lo+hi) per gather chunk
CALL_COLS = 8        # max columns (1024 idxs) per dma_gather call (SWDGE ring cap)
MLP_BLK = 4          # blocks per MLP chunk (512 nodes)
STEPS_DEV = STEPS    # override for bisection
PHASE = 4            # 1=AG only, 2=+gathers, 3=+reduce/diag, 4=full


def _gather_chunks(T_LO, T_HI):
    chunks = []
    b = 0
    while b < NB:
        e = b
        tot = 0
        while e < NB and tot + T_LO[e] + T_HI[e] <= GATHER_BUDGET:
            tot += T_LO[e] + T_HI[e]
            e += 1
        if e == b:
            e = b + 1  # single oversized block
        chunks.append((b, e))
        b = e
    return chunks


def build_program(lay):
    from concourse import bass, mybir, tile, bacc
    from concourse.masks import make_identity

    T_LO = [int(x) for x in lay["T_LO"]]
    T_HI = [int(x) for x in lay["T_HI"]]
    lo_base = [int(x) for x in lay["lo_base"]]
    hi_base = [int(x) for x in lay["hi_base"]]
    NLO, NHI = lay["NLO"], lay["NHI"]
    f32 = mybir.dt.float32
    i16 = mybir.dt.int16
    AL = mybir.AluOpType
    ACTF = mybir.ActivationFunctionType

    g_chunks = _gather_chunks(T_LO, T_HI)
    max_lo = max(sum(T_LO[b0:b1]) for b0, b1 in g_chunks)
    max_hi = max(sum(T_HI[b0:b1]) for b0, b1 in g_chunks)
    mlp_chunks = [(b, min(b + MLP_BLK, NB)) for b in range(0, NB, MLP_BLK)]

    nc = bacc.Bacc("TRN2", num_devices=N_CORES, debug=False)

    # --- I/O ---
    gat_t = nc.dram_tensor("gat_t", [128, CHP], f32, kind="ExternalInput")
    idx_lo_d = nc.dram_tensor("idx_lo", [128, NLO // 16], i16, kind="ExternalInput")
    idx_hi_d = nc.dram_tensor("idx_hi", [128, NHI // 16], i16, kind="ExternalInput")
    diag_d = nc.dram_tensor("diag", [NB * 128, 128], f32, kind="ExternalInput")
    win_d = nc.dram_tensor("win", [128, HID], f32, kind="ExternalInput")
    w1_d = nc.dram_tensor("w1", [128, 2 * HID], f32, kind="ExternalInput")
    w2_d = nc.dram_tensor("w2", [128, 2 * HID], f32, kind="ExternalInput")  # [K0|K1] halves side by side
    bin_d = nc.dram_tensor("bin", [128, 1], f32, kind="ExternalInput")
    b1_d = nc.dram_tensor("b1c", [128, 2], f32, kind="ExternalInput")
    b2_d = nc.dram_tensor("b2c", [128, 1], f32, kind="ExternalInput")
    ec_d = nc.dram_tensor("ec", [128, 2], f32, kind="ExternalInput")  # c1, dt/c1
    h_out = nc.dram_tensor("h_out", [CHP, HID], f32, kind="ExternalOutput")

    h_chunk_d = nc.dram_tensor("h_chunk", [CHP, HID], f32, kind="Internal")
    table = nc.dram_tensor("table", [TAB, HID], f32, kind="Internal")

    h_dram_ap = h_chunk_d[:].rearrange("(b p) f -> p b f", p=128)
    out_dram_ap = h_out[:].rearrange("(b p) f -> p b f", p=128)

    with tile.TileContext(nc) as tc:
        with (
            tc.tile_pool(name="const", bufs=1) as cpool,
            tc.tile_pool(name="gath", bufs=2) as gpool,
            tc.tile_pool(name="work", bufs=2) as wpool,
            tc.tile_pool(name="red", bufs=3) as rpool,
            tc.tile_pool(name="psum", bufs=2, space="PSUM") as pp,
            tc.tile_pool(name="psumy", bufs=3, space="PSUM") as ppy,
        ):
            # ---- resident constants
            idx_lo_sb = cpool.tile([128, NLO // 16], i16)
            idx_hi_sb = cpool.tile([128, NHI // 16], i16)
            diag_sb = cpool.tile([128, NB * 128], f32)
            win_sb = cpool.tile([128, HID], f32)
            w1_sb = cpool.tile([128, 2 * HID], f32)
            w2_sb = cpool.tile([128, 2 * HID], f32)
            bin_sb = cpool.tile([128, 1], f32)
            b1_sb = cpool.tile([128, 2], f32)
            b2_sb = cpool.tile([128, 1], f32)
            ec_sb = cpool.tile([128, 2], f32)
            ident = cpool.tile([128, 128], f32)
            h_sb = cpool.tile([128, NB * 128], f32)
            xt_sb = cpool.tile([128, NB * 128], f32)

            nc.sync.dma_start(idx_lo_sb[:], idx_lo_d[:])
            nc.sync.dma_start(idx_hi_sb[:], idx_hi_d[:])
            nc.sync.dma_start(
                diag_sb[:].rearrange("p (b f) -> p b f", f=128),
                diag_d[:].rearrange("(b p) f -> p b f", p=128),
            )
            nc.sync.dma_start(win_sb[:], win_d[:])
            nc.sync.dma_start(w1_sb[:], w1_d[:])
            nc.sync.dma_start(w2_sb[:], w2_d[:])
            nc.sync.dma_start(bin_sb[:], bin_d[:])
            nc.sync.dma_start(b1_sb[:], b1_d[:])
            nc.sync.dma_start(b2_sb[:], b2_d[:])
            nc.sync.dma_start(ec_sb[:], ec_d[:])
            make_identity(nc, ident[:])

            def mlp_block_tail(src_t, b0, b1, first):
                """transpose src_t (feature-major [128, n]) back to node rows
                and apply the Euler update (or plain copy when first=True)."""
                for b in range(b0, b1):
                    psd = pp.tile([128, 128], f32, tag="ps128")
                    nc.tensor.transpose(
                        psd[:], src_t[:, (b - b0) * 128:(b - b0 + 1) * 128], ident[:]
                    )
                    hslice = h_sb[:, b * 128:(b + 1) * 128]
                    if first:
                        nc.vector.tensor_copy(hslice, psd[:])
                    else:
                        u = rpool.tile([128, 128], f32, tag="u")
                        nc.vector.scalar_tensor_tensor(
                            u[:], psd[:], ec_sb[:, 1:2], hslice,
                            op0=AL.mult, op1=AL.add,
                        )
                        nc.vector.tensor_scalar_mul(hslice, u[:], ec_sb[:, 0:1])

            # ---- h0 = tanh(gat @ W_in + b_in)
            for b0, b1 in mlp_chunks:
                n = (b1 - b0) * 128
                gt = wpool.tile([128, n], f32, tag="gt")
                nc.sync.dma_start(gt[:], gat_t[:, b0 * 128:b1 * 128])
                psy = ppy.tile([128, n], f32, tag="psy")
                nc.tensor.matmul(
                    psy[:], lhsT=win_sb[:], rhs=gt[:],
                    start=True, stop=True,
                )
                h0t = wpool.tile([128, n], f32, tag="h0t")
                nc.scalar.activation(h0t[:], psy[:], ACTF.Tanh, bias=bin_sb[:, 0:1])
                mlp_block_tail(h0t, b0, b1, first=True)
            nc.vector.memset(h_sb[0:NPAD, 0:128], 0.0)
            nc.sync.dma_start(h_dram_ap, h_sb[:].rearrange("p (b f) -> p b f", f=128))

            # ---- Euler steps
            for s in range(STEPS_DEV):
                nc.gpsimd.collective_compute(
                    "AllGather", AL.bypass,
                    replica_groups=[list(range(N_CORES))],
                    ins=[h_chunk_d[:]], outs=[table[:]],
                )
                if PHASE < 2:
                    continue
                for b0, b1 in g_chunks:
                    nlo = sum(T_LO[b0:b1])
                    nhi = sum(T_HI[b0:b1])
                    glo = gpool.tile([128, max_lo, 128], f32, tag="glo")
                    ghi = gpool.tile([128, max_hi, 128], f32, tag="ghi")
                    for (ncols, gt, tab_ap, idxs, cbase) in (
                        (nlo, glo, table[:], idx_lo_sb, lo_base[b0]),
                        (nhi, ghi, table[HI_BASE:, :], idx_hi_sb, hi_base[b0]),
                    ):
                        for c0 in range(0, ncols, CALL_COLS):
                            cc = min(CALL_COLS, ncols - c0)
                            nc.gpsimd.dma_gather(
                                gt[:, c0:c0 + cc, :], tab_ap,
                                idxs[:, 8 * (cbase + c0): 8 * (cbase + c0 + cc)],
                                cc * 128, cc * 128, HID,
                            )
                    if PHASE < 3:
                        continue
                    for b in range(b0, b1):
                        reds = []
                        for (tile_g, base_arr, T_arr, coff) in (
                            (glo, lo_base, T_LO, lo_base[b0]),
                            (ghi, hi_base, T_HI, hi_base[b0]),
                        ):
                            T = T_arr[b]
                            if T == 0:
                                continue
                            a = base_arr[b] - coff
                            red = rpool.tile([128, 128], f32, tag="red")
                            nc.vector.tensor_reduce(
                                red[:],
                                tile_g[:, a:a + T, :].rearrange("p t f -> p f t"),
                                mybir.AxisListType.X, AL.add,
                            )
                            reds.append(red)
                        dg = diag_sb[:, b * 128:(b + 1) * 128]
                        psx = pp.tile([128, 128], f32, tag="ps128")
                        assert reds
                        for i, red in enumerate(reds):
                            nc.tensor.matmul(
                                psx[:], lhsT=red[:], rhs=dg,
                                start=(i == 0), stop=(i == len(reds) - 1),
                            )
                        nc.vector.tensor_copy(xt_sb[:, b * 128:(b + 1) * 128], psx[:])

                # MLP on X.T
                if PHASE < 4:
                    continue
                for b0, b1 in mlp_chunks:
                    n = (b1 - b0) * 128
                    xtr = xt_sb[:, b0 * 128:b1 * 128]
                    psy0 = ppy.tile([128, n], f32, tag="psy")
                    psy1 = ppy.tile([128, n], f32, tag="psy")
                    nc.tensor.matmul(psy0[:], lhsT=w1_sb[:, 0:HID],
                                     rhs=xtr, start=True, stop=True)
                    nc.tensor.matmul(psy1[:], lhsT=w1_sb[:, HID:2 * HID],
                                     rhs=xtr, start=True, stop=True)
                    h1a = wpool.tile([128, n], f32, tag="h1a")
                    h1b = wpool.tile([128, n], f32, tag="h1b")
                    nc.scalar.activation(h1a[:], psy0[:], ACTF.Gelu, bias=b1_sb[:, 0:1])
                    nc.scalar.activation(h1b[:], psy1[:], ACTF.Gelu, bias=b1_sb[:, 1:2])
                    psz = ppy.tile([128, n], f32, tag="psy")
                    nc.tensor.matmul(psz[:], lhsT=w2_sb[:, 0:HID],
                                     rhs=h1a[:], start=True, stop=False)
                    nc.tensor.matmul(psz[:], lhsT=w2_sb[:, HID:2 * HID],
                                     rhs=h1b[:], start=False, stop=True)
                    dft = wpool.tile([128, n], f32, tag="dft")
                    nc.scalar.activation(dft[:], psz[:], ACTF.Tanh, bias=b2_sb[:, 0:1])
                    mlp_block_tail(dft, b0, b1, first=False)

                nc.vector.memset(h_sb[0:NPAD, 0:128], 0.0)
                if s < STEPS_DEV - 1:
                    nc.sync.dma_start(
                        h_dram_ap, h_sb[:].rearrange("p (b f) -> p b f", f=128)
                    )
            nc.sync.dma_start(
                out_dram_ap, h_sb[:].rearrange("p (b f) -> p b f", f=128)
            )

    nc.compile()
    return nc


def make_in_maps(inputs, lay):
    gat_out = np.asarray(inputs["gat_out"], np.float32)
    W_in = np.asarray(inputs["W_in"], np.float32)
    b_in = np.asarray(inputs["b_in"], np.float32)
    W1 = np.asarray(inputs["W1"], np.float32)
    b1 = np.asarray(inputs["b1"], np.float32)
    W2 = np.asarray(inputs["W2"], np.float32)
    b2 = np.asarray(inputs["b2"], np.float32)
    clearance = np.asarray(inputs["clearance"], np.float32)

    decay = max(float(clearance[0]), 0.0)
    c1 = 1.0 - decay * DT
    ec = np.zeros((128, 2), np.float32)
    ec[:, 0] = c1
    ec[:, 1] = DT / c1

    w2cat = np.concatenate([W2[:HID, :], W2[HID:, :]], axis=1)  # [128, 256]
    b1c = b1.reshape(2, HID).T.copy()  # [128, 2]

    in_maps = []
    for c in range(N_CORES):
        lo, hi = c * CH, (c + 1) * CH
        gat_p = np.zeros((CHP, IN_DIM), np.float32)
        gat_p[lay["final_pos"][lo:hi]] = gat_out[lo:hi]
        diag = np.zeros((NB * 128, 128), np.float32)
        iv = lay["inv_deg"][c]
        for b in range(NB):
            blk = diag[b * 128:(b + 1) * 128]
            np.fill_diagonal(blk, iv[b * 128:(b + 1) * 128])
        in_maps.append({
            "gat_t": np.ascontiguousarray(gat_p.T),
            "idx_lo": lay["idx_lo"][c].reshape(-1, 16).T.copy(),
            "idx_hi": lay["idx_hi"][c].reshape(-1, 16).T.copy(),
            "diag": diag,
            "win": W_in,
            "w1": W1,
            "w2": w2cat,
            "bin": np.tile(b_in.reshape(HID, 1), (1, 1)),
            "b1c": b1c,
            "b2c": b2.reshape(HID, 1),
            "ec": ec,
        })
    # idx arrays: wrapped [16, n/16] -> replicate to 128 partitions
    for m in in_maps:
        m["idx_lo"] = np.tile(m["idx_lo"], (8, 1)).astype(np.int16)
        m["idx_hi"] = np.tile(m["idx_hi"], (8, 1)).astype(np.int16)
    return in_maps


_CACHE = {}


def kernel(**inputs):
    from concourse import bass_utils

    edge_key = hash(np.asarray(inputs["edge_index"]).tobytes())
    if edge_key not in _CACHE:
        lay = build_layout(inputs["edge_index"])
        nc = build_program(lay)
        _CACHE[edge_key] = (lay, nc)
    lay, nc = _CACHE[edge_key]

    in_maps = make_in_maps(inputs, lay)
    res = bass_utils.run_bass_kernel_spmd(
        nc, in_maps, core_ids=list(range(N_CORES))
    )
    out = np.zeros((N_NODES, HID), np.float32)
    for c in range(N_CORES):
        lo, hi = c * CH, (c + 1) * CH
        out[lo:hi] = res.results[c]["h_out"][lay["final_pos"][lo:hi]]
    return out


# ---------------------------------------------------------------------------
# numpy golden model of the device algorithm (for validating the layout)
# ---------------------------------------------------------------------------

def golden(inputs, lay):
    gat_out = np.asarray(inputs["gat_out"], np.float32)
    W_in = np.asarray(inputs["W_in"], np.float32)
    b_in = np.asarray(inputs["b_in"], np.float32)
    W1 = np.asarray(inputs["W1"], np.float32)
    b1 = np.asarray(inputs["b1"], np.float32)
    W2 = np.asarray(inputs["W2"], np.float32)
    b2 = np.asarray(inputs["b2"], np.float32)
    clearance = np.asarray(inputs["clearance"], np.float32)

    from scipy.special import erf

    def gelu(x):
        return x * 0.5 * (1.0 + erf(x / np.sqrt(2.0)))

    decay = max(clearance[0], 0.0)
    c1 = 1.0 - decay * DT

    # per-core padded gat chunks in final order
    gat_p = np.zeros((N_CORES, CHP, IN_DIM), np.float32)
    for c in range(N_CORES):
        lo, hi = c * CH, (c + 1) * CH
        gat_p[c, lay["final_pos"][lo:hi]] = gat_out[lo:hi]

    h = np.tanh(gat_p @ W_in + b_in)  # [C, CHP, H]; pad rows: tanh(0@W+0)=0
    h[:, :NPAD] = 0.0

    T_LO, T_HI = lay["T_LO"], lay["T_HI"]
    lo_base, hi_base = lay["lo_base"], lay["hi_base"]

    for _ in range(STEPS):
        table = h.reshape(TAB, HID)  # AllGather
        agg = np.zeros((N_CORES, CHP, HID), np.float32)
        for c in range(N_CORES):
            g_lo = table[lay["idx_lo"][c].astype(np.int64) + LO_BASE]
            g_hi = table[lay["idx_hi"][c].astype(np.int64) + HI_BASE]
            g_lo = g_lo.reshape(-1, 128, HID)  # [cols, 128, H] slot-major
            g_hi = g_hi.reshape(-1, 128, HID)
            for b in range(NB):
                s = g_lo[lo_base[b]:lo_base[b] + T_LO[b]].sum(0)
                s += g_hi[hi_base[b]:hi_base[b] + T_HI[b]].sum(0)
                agg[c, b * 128:(b + 1) * 128] = s
        x = agg * lay["inv_deg"][:, :, None]
        diff = np.tanh(gelu(x @ W1 + b1) @ W2 + b2)
        h = h * c1 + diff * DT
        h[:, :NPAD] = 0.0

    # reassemble
    out = np.zeros((N_NODES, HID), np.float32)
    for c in range(N_CORES):
        lo, hi = c * CH, (c + 1) * CH
        out[lo:hi] = h[c][lay["final_pos"][lo:hi]]
    return out

